# revision 9
# baseline (speedup 1.0000x reference)
"""Multi-Head Latent Attention (MLA) Trainium2 kernel, 8-way sharded.

Sharding: 8 cores = 2 (batch) x 4 (head groups of 4 heads).
Each core handles one batch element and 4 of the 16 heads.

v4 changes vs baseline (v1):
  - Query path folded on host: q[:, base|rope] = x @ (W_D_Q @ [W_U_Q_g | W_Q_R_g]),
    eliminating the duplicated qc = x @ W_D_Q latent (DCQ=1536) per core.
  - All projection weights SBUF-resident, loaded once (v1 re-streamed them
    every token chunk: ~104MB -> ~18MB weight traffic per core).
  - q/k/v spills to DRAM in bf16 (half traffic); attention + output matmuls
    run in bf16 (same PE rate as f32r: 1 cyc/row), accumulating in f32 PSUM.
  - Softmax denominators: P^T tiles accumulated on DVE into pt_acc, then ONE
    ones-matmul per (head, query-block) instead of one per key-tile.
  - DMA batching: real HWDGE has multi-us fixed cost per dma_start and
    serializes transfers, so everything moves in few large instructions
    (3-D APs): weights packed host-side into 6 loads, 1 x-load per chunk,
    1 write per spill tensor per chunk, per-head q loads, 4 output writes.

Everything is computed TRANSPOSED (feature dim on partitions): scores come
out as S^T (keys on partitions, queries free), softmax = plain exp (scores
O(+-6)), no on-chip transposes; RoPE pair-swap is a small constant matmul
plus elementwise mul/add.
"""

import sys

sys.path.insert(0, "/opt/trn_rl_repo")

import numpy as np
import ml_dtypes

import concourse.bacc as bacc
import concourse.mybir as mybir
import concourse.tile as tile
from concourse.bass_utils import run_bass_kernel_spmd

# Problem dims (hardcoded per contract)
D, NH, DH, DC, DCQ, DHR = 2048, 16, 128, 512, 1536, 64
B, L = 2, 2048
ROPE_THETA = 10000.0

NHG = 4                 # heads per core
DQB = NHG * DH          # 512: per-core base q/k feature dim (also v dim)
DQR = NHG * DHR         # 256: per-core rope feature dim
DQ = DQB + DQR          # 768: folded q feature dim
P = 128
CW = 256                # phase-A token chunk width
SCALE = DH ** -0.5

F32R = mybir.dt.float32r
F32 = mybir.dt.float32
BF16 = mybir.dt.bfloat16

_CACHED = {}


def _build(repeat=None):
    """Build the SPMD program. repeat=N wraps the body in a HW loop (for
    perf measurement only — amortizes host dispatch overhead)."""
    nc = bacc.Bacc("TRN2", target_bir_lowering=False, debug=False)

    # ---- DRAM I/O (per-core data; program is SPMD)
    xT = nc.dram_tensor("xT", [D, L], F32R, kind="ExternalInput")
    wdv = nc.dram_tensor("wdv", [D, DC], F32R, kind="ExternalInput")
    wkrd = nc.dram_tensor("wkrd", [D, DQR], F32R, kind="ExternalInput")
    weff = nc.dram_tensor("weff", [D, DQ], F32R, kind="ExternalInput")
    wukv = nc.dram_tensor("wukv", [DC, 2 * DQB], F32R, kind="ExternalInput")
    wo = nc.dram_tensor("wo", [DQB, D], BF16, kind="ExternalInput")
    csd = nc.dram_tensor("csd", [P, 2 * L], F32, kind="ExternalInput")
    pod = nc.dram_tensor("pod", [P, 2 * P], F32R, kind="ExternalInput")
    out = nc.dram_tensor("out", [L, D], F32, kind="ExternalOutput")

    # ---- internal DRAM spill (transposed q/k, natural v) — bf16
    qbT_d = nc.dram_tensor("qbT_d", [DQB, L], BF16)
    qrT_d = nc.dram_tensor("qrT_d", [DQR, L], BF16)
    kbT_d = nc.dram_tensor("kbT_d", [DQB, L], BF16)
    krT_d = nc.dram_tensor("krT_d", [DQR, L], BF16)
    v_d = nc.dram_tensor("v_d", [L, DQB], BF16)

    KD = D // P      # 16
    KC = DC // P     # 4
    NCH = L // CW    # chunks

    # 3-D views for single-DMA multi-tile transfers
    qbT_3 = qbT_d.rearrange("(m p) j -> p m j", p=P)   # [P, 4, L]
    qrT_3 = qrT_d.rearrange("(m p) j -> p m j", p=P)   # [P, 2, L]
    kbT_3 = kbT_d.rearrange("(m p) j -> p m j", p=P)   # [P, 4, L]
    krT_3 = krT_d.rearrange("(m p) j -> p m j", p=P)   # [P, 2, L]
    v_3 = v_d.rearrange("(lt p) j -> p lt j", p=P)     # [P, 16, DQB]
    xT_3 = xT.rearrange("(k p) j -> p k j", p=P)       # [P, 16, L]
    wdv_3 = wdv.rearrange("(k p) j -> p k j", p=P)     # [P, 16, DC]
    wkrd_3 = wkrd.rearrange("(k p) j -> p k j", p=P)   # [P, 16, DQR]
    weff_3 = weff.rearrange("(k p) j -> p k j", p=P)   # [P, 16, DQ]
    wukv_3 = wukv.rearrange("(k p) j -> p k j", p=P)   # [P, 4, 2*DQB]
    wo_3 = wo.rearrange("(k p) j -> p k j", p=P)       # [P, 4, D]
    out_3 = out.rearrange("(g p) j -> p g j", p=P)     # [P, 16, D]

    from contextlib import nullcontext
    with tile.TileContext(nc) as tc:
        with (tc.For_i(0, repeat, 1) if repeat else nullcontext()), \
             tc.tile_pool(name="constp", bufs=1) as constp, \
             tc.tile_pool(name="otp_res", bufs=1) as otp_res:
            oT_res = [otp_res.tile([P, L], BF16, name=f"oT{h}", tag=f"oT{h}")
                      for h in range(NHG)]

            # ================= Phase A: projections (token-chunked) =========
            with tc.tile_pool(name="wres", bufs=1) as wres, \
                 tc.tile_pool(name="xp", bufs=2) as xp, \
                 tc.tile_pool(name="ctp", bufs=6) as ctp, \
                 tc.tile_pool(name="rop", bufs=4) as rop, \
                 tc.tile_pool(name="evp", bufs=2) as evp, \
                 tc.tile_pool(name="rtmp", bufs=2) as rtmp, \
                 tc.tile_pool(name="psA", bufs=6, space="PSUM") as psA:

                def load_x(ch):
                    xt3 = xp.tile([P, KD, CW], F32R, name="xt3", tag="xt3")
                    nc.sync.dma_start(
                        out=xt3[:],
                        in_=xT_3[:, :, ch * CW:(ch + 1) * CW])
                    return xt3

                # chunk-0 activations first, then resident weights in order of
                # first use — compute starts as soon as x(0)+wdv land instead
                # of waiting out the whole weight preload.
                xt3_0 = load_x(0)
                wdv_t = wres.tile([P, KD, DC], F32R, name="wdv_t", tag="wdv")
                nc.sync.dma_start(out=wdv_t[:], in_=wdv_3[:])
                wukv_t = wres.tile([P, KC, 2 * DQB], F32R, name="wukv_t", tag="wukv")
                nc.sync.dma_start(out=wukv_t[:], in_=wukv_3[:])
                wkr_t = wres.tile([P, KD, DQR], F32R, name="wkr_t", tag="wkr")
                nc.sync.dma_start(out=wkr_t[:], in_=wkrd_3[:])
                # cos|sin full-L (f32) and prot|ones (f32 bits into f32r tile)
                cst = wres.tile([P, 2 * L], F32, name="cst", tag="cst")
                nc.sync.dma_start(out=cst[:], in_=csd[:, :])
                po_t = constp.tile([P, 2 * P], F32R, name="po_t", tag="po")
                nc.sync.dma_start(out=po_t[:], in_=pod[:, :])
                weff_t = wres.tile([P, KD, DQ], F32R, name="weff_t", tag="weff")
                nc.sync.dma_start(out=weff_t[:], in_=weff_3[:])

                prot_t = po_t[:, 0:P]
                ones_t = po_t[:, P:2 * P]

                def mm_acc(ps, wt3, coff, xt3, nk):
                    for k in range(nk):
                        nc.tensor.matmul(
                            ps[:], wt3[:, k, coff:coff + P], xt3[:, k, :],
                            start=(k == 0), stop=(k == nk - 1))

                for ch in range(NCH):
                    tsl = slice(ch * CW, (ch + 1) * CW)
                    xt3 = xt3_0 if ch == 0 else load_x(ch)

                    # c^T slab (DC x CW), kept f32r for kb/v matmuls
                    cts = []
                    for m in range(KC):
                        ct = ctp.tile([P, CW], F32R, name="ct", tag="ct")
                        ps = psA.tile([P, CW], F32, name="ps_c", tag="psa")
                        mm_acc(ps, wdv_t, m * P, xt3, KD)
                        nc.any.tensor_copy(ct[:], ps[:])
                        cts.append(ct)

                    # k_base^T (DQB x CW) -> one bf16 spill write
                    kb3 = evp.tile([P, DQB // P, CW], BF16, name="kb3", tag="kb3")
                    for m in range(DQB // P):
                        ps = psA.tile([P, CW], F32, name="ps_kb", tag="psa")
                        for k in range(KC):
                            nc.tensor.matmul(
                                ps[:], wukv_t[:, k, m * P:(m + 1) * P], cts[k][:],
                                start=(k == 0), stop=(k == KC - 1))
                        nc.any.tensor_copy(kb3[:, m, :], ps[:])
                    nc.sync.dma_start(out=kbT_3[:, :, tsl], in_=kb3[:])

                    # v natural (CW tokens x DQB) -> one bf16 spill write
                    v3 = evp.tile([P, CW // P, DQB], BF16, name="v3", tag="v3")
                    for lt in range(CW // P):
                        ps = psA.tile([P, DQB], F32, name="ps_v", tag="psa")
                        for k in range(KC):
                            nc.tensor.matmul(
                                ps[:], cts[k][:, lt * P:(lt + 1) * P],
                                wukv_t[:, k, DQB:2 * DQB],
                                start=(k == 0), stop=(k == KC - 1))
                        nc.any.tensor_copy(v3[:, lt, :], ps[:])
                    nc.sync.dma_start(
                        out=v_3[:, ch * (CW // P):(ch + 1) * (CW // P), :],
                        in_=v3[:])

                    # k_rope^T raw (DQR x CW) — held for RoPE below
                    krts = []
                    for m in range(DQR // P):
                        krt = rop.tile([P, CW], F32R, name="krt", tag="rop")
                        ps = psA.tile([P, CW], F32, name="ps_kr", tag="psa")
                        mm_acc(ps, wkr_t, m * P, xt3, KD)
                        nc.any.tensor_copy(krt[:], ps[:])
                        krts.append(krt)

                    # folded q^T (DQ x CW): m 0..3 = base -> spill, 4..5 = rope raw
                    qrts = []
                    qb3 = evp.tile([P, DQB // P, CW], BF16, name="qb3", tag="qb3")
                    for m in range(DQ // P):
                        ps = psA.tile([P, CW], F32, name="ps_q", tag="psa")
                        mm_acc(ps, weff_t, m * P, xt3, KD)
                        if m < DQB // P:
                            nc.any.tensor_copy(qb3[:, m, :], ps[:])
                        else:
                            qrt = rop.tile([P, CW], F32R, name="qrt", tag="rop")
                            nc.any.tensor_copy(qrt[:], ps[:])
                            qrts.append(qrt)
                    nc.sync.dma_start(out=qbT_3[:, :, tsl], in_=qb3[:])

                    # RoPE: final = cos (.) raw + sin (.) (Prot @ raw) -> bf16
                    cos_sl = cst[:, ch * CW:(ch + 1) * CW]
                    sin_sl = cst[:, L + ch * CW: L + (ch + 1) * CW]
                    for raws, dst3, ftag in ((qrts, qrT_3, "fq"), (krts, krT_3, "fk")):
                        f3 = evp.tile([P, DQR // P, CW], BF16, name=ftag, tag=ftag)
                        for m, raw in enumerate(raws):
                            rps = psA.tile([P, CW], F32, name="rps", tag="rps", bufs=2)
                            nc.tensor.matmul(rps[:], prot_t, raw[:],
                                             start=True, stop=True)
                            t1 = rtmp.tile([P, CW], F32, name="t1", tag="t1")
                            nc.any.tensor_mul(t1[:], cos_sl, raw[:])
                            t2 = rtmp.tile([P, CW], F32, name="t2", tag="t2")
                            nc.any.tensor_mul(t2[:], sin_sl, rps[:])
                            nc.any.tensor_add(f3[:, m, :], t1[:], t2[:])
                        nc.sync.dma_start(out=dst3[:, :, tsl], in_=f3[:])

            # ================= Phase B: attention (bf16) ====================
            LQ = 512
            with tc.tile_pool(name="wop", bufs=1) as wop:
              wot3 = wop.tile([P, NHG, D], BF16, name="wot3", tag="wo")
              with tc.tile_pool(name="khp", bufs=2) as khp, \
                 tc.tile_pool(name="vhp", bufs=2) as vhp, \
                 tc.tile_pool(name="qlq", bufs=2) as qlqp, \
                 tc.tile_pool(name="ptp", bufs=4) as ptp, \
                 tc.tile_pool(name="pap", bufs=2) as pap, \
                 tc.tile_pool(name="rcp", bufs=2) as rcp, \
                 tc.tile_pool(name="stp", bufs=3, space="PSUM") as stp, \
                 tc.tile_pool(name="otp", bufs=2, space="PSUM") as otp, \
                 tc.tile_pool(name="rsp", bufs=2, space="PSUM") as rsp:
                for h in range(NHG):
                    kb_h = khp.tile([P, L], BF16, name="kb_h", tag="kb")
                    nc.sync.dma_start(out=kb_h[:], in_=kbT_d[h * P:(h + 1) * P, :])
                    kr_h = khp.tile([DHR, L], BF16, name="kr_h", tag="kr")
                    nc.sync.dma_start(out=kr_h[:], in_=krT_d[h * DHR:(h + 1) * DHR, :])
                    # all 16 (128x128) V k-tiles for this head in one DMA
                    v_h = vhp.tile([P, L // P, P], BF16, name="v_h", tag="vh")
                    nc.sync.dma_start(
                        out=v_h[:],
                        in_=v_d[:, h * DH:(h + 1) * DH].rearrange(
                            "(lk p) j -> p lk j", p=P))
                    vts = [v_h[:, lk, :] for lk in range(L // P)]
                    # full-row q for this head (one DMA each)
                    qb_h = qlqp.tile([P, L], BF16, name="qb_h", tag="qb")
                    nc.sync.dma_start(out=qb_h[:], in_=qbT_d[h * P:(h + 1) * P, :])
                    qr_h = qlqp.tile([DHR, L], BF16, name="qr_h", tag="qr")
                    nc.sync.dma_start(out=qr_h[:], in_=qrT_d[h * DHR:(h + 1) * DHR, :])
                    if h == 0:
                        # prefetch W_O (bf16 from host) during attention
                        nc.sync.dma_start(out=wot3[:], in_=wo_3[:])
                    for lq in range(L // LQ):
                        qsl = slice(lq * LQ, (lq + 1) * LQ)
                        ot_ps = otp.tile([P, LQ], F32, name="ot_ps", tag="ot")
                        pt_acc = pap.tile([P, LQ], F32R, name="pt_acc", tag="pa")
                        for lk in range(L // P):
                            st_ps = stp.tile([P, LQ], F32, name="st_ps", tag="st")
                            nc.tensor.matmul(
                                st_ps[:], kb_h[:, lk * P:(lk + 1) * P],
                                qb_h[:, qsl], start=True, stop=False)
                            nc.tensor.matmul(
                                st_ps[:], kr_h[:, lk * P:(lk + 1) * P],
                                qr_h[:, qsl], start=False, stop=True)
                            pt = ptp.tile([P, LQ], BF16, name="pt", tag="pt")
                            nc.scalar.activation(
                                pt[:], st_ps[:], mybir.ActivationFunctionType.Exp,
                                scale=SCALE)
                            nc.tensor.matmul(
                                ot_ps[:], vts[lk][:], pt[:],
                                start=(lk == 0), stop=(lk == L // P - 1))
                            if lk == 0:
                                nc.any.tensor_copy(pt_acc[:], pt[:])
                            else:
                                nc.any.tensor_add(pt_acc[:], pt_acc[:], pt[:])
                        rs_ps = rsp.tile([P, LQ], F32, name="rs_ps", tag="rs")
                        nc.tensor.matmul(rs_ps[:], ones_t, pt_acc[:],
                                         start=True, stop=True)
                        rec = rcp.tile([P, LQ], F32, name="rec", tag="rec")
                        nc.vector.reciprocal(rec[:], rs_ps[:])
                        nc.any.tensor_mul(oT_res[h][:, qsl], ot_ps[:], rec[:])

              # ============= Phase C: output projection (bf16) ============
              with tc.tile_pool(name="ocp", bufs=2) as ocp, \
                   tc.tile_pool(name="psC", bufs=4, space="PSUM") as psC:
                  for st4 in range(L // P // 4):
                      oc = ocp.tile([P, 4, D], F32, name="oc", tag="oc")
                      for g in range(4):
                          mt = st4 * 4 + g
                          for nt in range(D // 512):
                              ps = psC.tile([P, 512], F32, name="ps_o", tag="psc")
                              for k in range(NHG):
                                  nc.tensor.matmul(
                                      ps[:], oT_res[k][:, mt * P:(mt + 1) * P],
                                      wot3[:, k, nt * 512:(nt + 1) * 512],
                                      start=(k == 0), stop=(k == NHG - 1))
                              nc.any.tensor_copy(
                                  oc[:, g, nt * 512:(nt + 1) * 512], ps[:])
                      nc.sync.dma_start(
                          out=out_3[:, st4 * 4:(st4 + 1) * 4, :], in_=oc[:])

    nc.compile()
    return nc


def _rope_tables():
    """cos/sin in transposed, 2-head-replicated layout (128 x L), plus Prot^T."""
    inv_freq = 1.0 / (ROPE_THETA ** (np.arange(0, DHR, 2, dtype=np.float32) / DHR))
    ang = np.arange(L, dtype=np.float32)[:, None] * inv_freq[None, :]  # (L, 32)
    cos64 = np.concatenate([np.cos(ang), np.cos(ang)], axis=1).T  # (64, L)
    sin64 = np.concatenate([np.sin(ang), np.sin(ang)], axis=1).T
    cosr = np.ascontiguousarray(np.tile(cos64, (2, 1)), dtype=np.float32)
    sinr = np.ascontiguousarray(np.tile(sin64, (2, 1)), dtype=np.float32)
    # rot(x) = [-x2, x1] per 64-dim head: Prot rows 0:32 = -I at cols 32:64,
    # rows 32:64 = +I at cols 0:32; block-diag over 2 heads; pass transposed.
    p64 = np.zeros((DHR, DHR), dtype=np.float32)
    half = DHR // 2
    p64[np.arange(half), np.arange(half) + half] = -1.0
    p64[np.arange(half) + half, np.arange(half)] = 1.0
    p128 = np.zeros((P, P), dtype=np.float32)
    p128[:DHR, :DHR] = p64
    p128[DHR:, DHR:] = p64
    protT = np.ascontiguousarray(p128.T)
    return cosr, sinr, protT


def _make_in_maps(inputs):
    """Build the 8 per-core input maps from the full-problem input dict."""
    cosr, sinr, protT = _rope_tables()
    f = np.float32
    bf = ml_dtypes.bfloat16
    x = np.asarray(inputs["x"])
    xTs = [np.ascontiguousarray(x[b].T, dtype=f) for b in range(B)]
    # host-side query-path fold (float64 for a clean compose, cast to f32)
    wdq = np.asarray(inputs["W_D_Q"], np.float64)
    weffb_all = (wdq @ np.asarray(inputs["W_U_Q"], np.float64)).astype(f)
    weffr_all = (wdq @ np.asarray(inputs["W_Q_R"], np.float64)).astype(f)
    W_D_KV = np.ascontiguousarray(inputs["W_D_KV"], dtype=f)
    csd = np.ascontiguousarray(np.concatenate([cosr, sinr], axis=1), dtype=f)
    pod = np.ascontiguousarray(np.concatenate(
        [protT.astype(np.float32), np.ones((P, P), np.float32)], axis=1), dtype=f)
    in_maps = []
    for c in range(8):
        b, g = c // 4, c % 4
        hb = slice(g * DQB, (g + 1) * DQB)
        hr = slice(g * DQR, (g + 1) * DQR)
        weff_c = np.concatenate([weffb_all[:, hb], weffr_all[:, hr]], axis=1)
        wukv_c = np.concatenate(
            [np.asarray(inputs["W_U_K"])[:, hb], np.asarray(inputs["W_U_V"])[:, hb]],
            axis=1)
        in_maps.append(dict(
            xT=xTs[b],
            wdv=W_D_KV,
            wkrd=np.ascontiguousarray(np.asarray(inputs["W_K_R"])[:, hr], dtype=f),
            weff=np.ascontiguousarray(weff_c, dtype=f),
            wukv=np.ascontiguousarray(wukv_c, dtype=f),
            wo=np.ascontiguousarray(np.asarray(inputs["W_O"])[hb, :]).astype(bf),
            csd=csd, pod=pod,
        ))
    return in_maps


def kernel(x, W_D_Q, W_U_Q, W_Q_R, W_D_KV, W_U_K, W_K_R, W_U_V, W_O):
    if "nc" not in _CACHED:
        _CACHED["nc"] = _build()
    nc = _CACHED["nc"]

    in_maps = _make_in_maps(dict(
        x=x, W_D_Q=W_D_Q, W_U_Q=W_U_Q, W_Q_R=W_Q_R, W_D_KV=W_D_KV,
        W_U_K=W_U_K, W_K_R=W_K_R, W_U_V=W_U_V, W_O=W_O))
    res = run_bass_kernel_spmd(nc, in_maps, core_ids=list(range(8)))
    outs = [r["out"] for r in res.results]
    full = np.stack(
        [outs[b * 4] + outs[b * 4 + 1] + outs[b * 4 + 2] + outs[b * 4 + 3]
         for b in range(B)]).astype(np.float32)
    return full


# revision 17
# speedup vs baseline: 1.1258x; 1.1258x over previous
"""Multi-Head Latent Attention (MLA) Trainium2 kernel, 8-way sharded.

Sharding: 8 cores = 2 (batch) x 4 (head groups of 4 heads).
Each core handles one batch element and 4 of the 16 heads.

v4 changes vs baseline (v1):
  - Query path folded on host: q[:, base|rope] = x @ (W_D_Q @ [W_U_Q_g | W_Q_R_g]),
    eliminating the duplicated qc = x @ W_D_Q latent (DCQ=1536) per core.
  - All projection weights SBUF-resident, loaded once (v1 re-streamed them
    every token chunk: ~104MB -> ~18MB weight traffic per core).
  - q/k/v spills to DRAM in bf16 (half traffic); attention + output matmuls
    run in bf16 (same PE rate as f32r: 1 cyc/row), accumulating in f32 PSUM.
  - Softmax denominators: P^T tiles accumulated on DVE into pt_acc, then ONE
    ones-matmul per (head, query-block) instead of one per key-tile.
  - DMA batching: real HWDGE has multi-us fixed cost per dma_start and
    serializes transfers, so everything moves in few large instructions
    (3-D APs): weights packed host-side into 6 loads, 1 x-load per chunk,
    1 write per spill tensor per chunk, per-head q loads, 4 output writes.

Everything is computed TRANSPOSED (feature dim on partitions): scores come
out as S^T (keys on partitions, queries free), softmax = plain exp (scores
O(+-6)), no on-chip transposes; RoPE pair-swap is a small constant matmul
plus elementwise mul/add.
"""

import sys

sys.path.insert(0, "/opt/trn_rl_repo")

import numpy as np
import ml_dtypes

import concourse.bacc as bacc
import concourse.mybir as mybir
import concourse.tile as tile
from concourse.bass_utils import run_bass_kernel_spmd

# Problem dims (hardcoded per contract)
D, NH, DH, DC, DCQ, DHR = 2048, 16, 128, 512, 1536, 64
B, L = 2, 2048
ROPE_THETA = 10000.0

NHG = 4                 # heads per core
DQB = NHG * DH          # 512: per-core base q/k feature dim (also v dim)
DQR = NHG * DHR         # 256: per-core rope feature dim
DQ = DQB + DQR          # 768: folded q feature dim
P = 128
CW = 512                # phase-A token chunk width
SCALE = DH ** -0.5

F32R = mybir.dt.float32r
F32 = mybir.dt.float32
BF16 = mybir.dt.bfloat16

_CACHED = {}


def _build(repeat=None):
    """Build the SPMD program. repeat=N wraps the body in a HW loop (for
    perf measurement only — amortizes host dispatch overhead)."""
    nc = bacc.Bacc("TRN2", target_bir_lowering=False, debug=False)

    # ---- DRAM I/O (per-core data; program is SPMD)
    xT = nc.dram_tensor("xT", [D, L], BF16, kind="ExternalInput")
    wdv = nc.dram_tensor("wdv", [D, DC], BF16, kind="ExternalInput")
    wkrd = nc.dram_tensor("wkrd", [D, DQR], BF16, kind="ExternalInput")
    weff = nc.dram_tensor("weff", [D, DQ], BF16, kind="ExternalInput")
    wukv = nc.dram_tensor("wukv", [DC, 2 * DQB], BF16, kind="ExternalInput")
    wo = nc.dram_tensor("wo", [DQB, D], BF16, kind="ExternalInput")
    csd = nc.dram_tensor("csd", [P, 2 * L], F32, kind="ExternalInput")
    pod = nc.dram_tensor("pod", [P, 2 * P], F32R, kind="ExternalInput")
    out = nc.dram_tensor("out", [L, D], BF16, kind="ExternalOutput")

    KD = D // P      # 16
    KC = DC // P     # 4
    NCH = L // CW    # chunks

    # 3-D views for single-DMA multi-tile transfers
    xT_3 = xT.rearrange("(k p) j -> p k j", p=P)       # [P, 16, L]
    wdv_3 = wdv.rearrange("(k p) j -> p k j", p=P)     # [P, 16, DC]
    wkrd_3 = wkrd.rearrange("(k p) j -> p k j", p=P)   # [P, 16, DQR]
    weff_3 = weff.rearrange("(k p) j -> p k j", p=P)   # [P, 16, DQ]
    wukv_3 = wukv.rearrange("(k p) j -> p k j", p=P)   # [P, 4, 2*DQB]
    wo_3 = wo.rearrange("(k p) j -> p k j", p=P)       # [P, 4, D]
    out_3 = out.rearrange("(g p) j -> p g j", p=P)     # [P, 16, D]

    from contextlib import nullcontext
    with tile.TileContext(nc) as tc:
        with (tc.For_i(0, repeat, 1) if repeat else nullcontext()), \
             tc.tile_pool(name="constp", bufs=1) as constp, \
             tc.tile_pool(name="otp_res", bufs=1) as otp_res:
            oT_res = [otp_res.tile([P, L], BF16, name=f"oT{h}", tag=f"oT{h}")
                      for h in range(NHG)]
            kb_res = [otp_res.tile([P, L], BF16, name=f"kb{h}", tag=f"kb{h}")
                      for h in range(NHG)]
            kr_res = [otp_res.tile([P, L], BF16, name=f"kr{m}", tag=f"kr{m}")
                      for m in range(DQR // P)]
            v_res = otp_res.tile([P, L // P, DQB], BF16, name="v_res", tag="v_res")
            qb_res = [otp_res.tile([P, L], BF16, name=f"qb{h}", tag=f"qb{h}")
                      for h in range(NHG)]
            qr_res = [otp_res.tile([P, L], BF16, name=f"qr{m}", tag=f"qr{m}")
                      for m in range(DQR // P)]

            # ================= Phase A: projections (token-chunked) =========
            with tc.tile_pool(name="wres", bufs=1) as wres, \
                 tc.tile_pool(name="xp", bufs=2) as xp, \
                 tc.tile_pool(name="ctp", bufs=6) as ctp, \
                 tc.tile_pool(name="rop", bufs=4) as rop, \
                 tc.tile_pool(name="evp", bufs=2) as evp, \
                 tc.tile_pool(name="rtmp", bufs=2) as rtmp, \
                 tc.tile_pool(name="psA", bufs=6, space="PSUM") as psA:

                def load_x(ch):
                    xt3 = xp.tile([P, KD, CW], BF16, name="xt3", tag="xt3")
                    nc.sync.dma_start(
                        out=xt3[:],
                        in_=xT_3[:, :, ch * CW:(ch + 1) * CW])
                    return xt3

                # chunk-0 activations first, then resident weights in order of
                # first use — compute starts as soon as x(0)+wdv land instead
                # of waiting out the whole weight preload.
                xt3_0 = load_x(0)
                wdv_t = wres.tile([P, KD, DC], BF16, name="wdv_t", tag="wdv")
                nc.sync.dma_start(out=wdv_t[:, :, : DC // 2],
                                  in_=wdv_3[:, :, : DC // 2])
                nc.sync.dma_start(out=wdv_t[:, :, DC // 2:],
                                  in_=wdv_3[:, :, DC // 2:])
                wukv_t = wres.tile([P, KC, 2 * DQB], BF16, name="wukv_t", tag="wukv")
                nc.sync.dma_start(out=wukv_t[:], in_=wukv_3[:])
                wkr_t = wres.tile([P, KD, DQR], BF16, name="wkr_t", tag="wkr")
                nc.sync.dma_start(out=wkr_t[:], in_=wkrd_3[:])
                # weff split in two halves so chunk-0 q matmuls start after
                # the first half lands instead of the full 6.3MB
                weff_t = wres.tile([P, KD, DQ], BF16, name="weff_t", tag="weff")
                nc.sync.dma_start(out=weff_t[:, :, : DQ // 2],
                                  in_=weff_3[:, :, : DQ // 2])
                nc.sync.dma_start(out=weff_t[:, :, DQ // 2:],
                                  in_=weff_3[:, :, DQ // 2:])
                # prot|ones (f32 bits into f32r tile)
                po_t = constp.tile([P, 2 * P], F32R, name="po_t", tag="po")
                nc.sync.dma_start(out=po_t[:], in_=pod[:, :])
                csd_3 = csd.rearrange("p (s j) -> p s j", s=2)  # [P, 2, L]

                prot_t = po_t[:, 0:P]
                ones_t = po_t[:, P:2 * P]

                def mm_acc(ps, wt3, coff, xt3, nk):
                    for k in range(nk):
                        nc.tensor.matmul(
                            ps[:], wt3[:, k, coff:coff + P], xt3[:, k, :],
                            start=(k == 0), stop=(k == nk - 1))

                xts_pref = {0: xt3_0}
                for ch in range(NCH):
                    tsl = slice(ch * CW, (ch + 1) * CW)
                    xt3 = xts_pref.pop(ch)
                    # prefetch next chunk's x ahead of this chunk's spill writes
                    if ch + 1 < NCH:
                        xts_pref[ch + 1] = load_x(ch + 1)

                    # c^T slab (DC x CW), kept f32r for kb/v matmuls
                    cts = []
                    for m in range(KC):
                        ct = ctp.tile([P, CW], BF16, name="ct", tag="ct")
                        ps = psA.tile([P, CW], F32, name="ps_c", tag="psa")
                        mm_acc(ps, wdv_t, m * P, xt3, KD)
                        nc.any.tensor_copy(ct[:], ps[:])
                        cts.append(ct)

                    # k_base^T (DQB x CW) -> straight into resident tiles
                    for m in range(DQB // P):
                        ps = psA.tile([P, CW], F32, name="ps_kb", tag="psa")
                        for k in range(KC):
                            nc.tensor.matmul(
                                ps[:], wukv_t[:, k, m * P:(m + 1) * P], cts[k][:],
                                start=(k == 0), stop=(k == KC - 1))
                        nc.any.tensor_copy(kb_res[m][:, tsl], ps[:])

                    # v natural (CW tokens x DQB) -> straight into resident
                    for lt in range(CW // P):
                        ps = psA.tile([P, DQB], F32, name="ps_v", tag="psa")
                        for k in range(KC):
                            nc.tensor.matmul(
                                ps[:], cts[k][:, lt * P:(lt + 1) * P],
                                wukv_t[:, k, DQB:2 * DQB],
                                start=(k == 0), stop=(k == KC - 1))
                        nc.any.tensor_copy(v_res[:, ch * (CW // P) + lt, :], ps[:])

                    # k_rope^T raw (DQR x CW) — held for RoPE below
                    krts = []
                    for m in range(DQR // P):
                        krt = rop.tile([P, CW], F32R, name="krt", tag="rop")
                        ps = psA.tile([P, CW], F32, name="ps_kr", tag="psa")
                        mm_acc(ps, wkr_t, m * P, xt3, KD)
                        nc.any.tensor_copy(krt[:], ps[:])
                        krts.append(krt)

                    # folded q^T (DQ x CW): m 0..3 = base -> resident, 4..5 = rope
                    qrts = []
                    for m in range(DQ // P):
                        ps = psA.tile([P, CW], F32, name="ps_q", tag="psa")
                        mm_acc(ps, weff_t, m * P, xt3, KD)
                        if m < DQB // P:
                            nc.any.tensor_copy(qb_res[m][:, tsl], ps[:])
                        else:
                            qrt = rop.tile([P, CW], F32R, name="qrt", tag="rop")
                            nc.any.tensor_copy(qrt[:], ps[:])
                            qrts.append(qrt)

                    # RoPE: final = cos (.) raw + sin (.) (Prot @ raw) -> bf16
                    cs_t = rtmp.tile([P, 2, CW], F32, name="cs_t", tag="cs", bufs=2)
                    nc.sync.dma_start(out=cs_t[:], in_=csd_3[:, :, tsl])
                    cos_sl = cs_t[:, 0, :]
                    sin_sl = cs_t[:, 1, :]
                    for m, raw in enumerate(qrts):
                        rps = psA.tile([P, CW], F32, name="rps", tag="rps", bufs=2)
                        nc.tensor.matmul(rps[:], prot_t, raw[:],
                                         start=True, stop=True)
                        t1 = rtmp.tile([P, CW], F32, name="t1", tag="t1")
                        nc.any.tensor_mul(t1[:], cos_sl, raw[:])
                        t2 = rtmp.tile([P, CW], F32, name="t2", tag="t2")
                        nc.any.tensor_mul(t2[:], sin_sl, rps[:])
                        nc.any.tensor_add(qr_res[m][:, tsl], t1[:], t2[:])
                    for m, raw in enumerate(krts):
                        rps = psA.tile([P, CW], F32, name="rps", tag="rps", bufs=2)
                        nc.tensor.matmul(rps[:], prot_t, raw[:],
                                         start=True, stop=True)
                        t1 = rtmp.tile([P, CW], F32, name="t1", tag="t1")
                        nc.any.tensor_mul(t1[:], cos_sl, raw[:])
                        t2 = rtmp.tile([P, CW], F32, name="t2", tag="t2")
                        nc.any.tensor_mul(t2[:], sin_sl, rps[:])
                        nc.any.tensor_add(kr_res[m][:, tsl], t1[:], t2[:])

            # ================= Phase B: attention (bf16) ====================
            LQ = 512
            with tc.tile_pool(name="wop", bufs=1) as wop:
              wot3 = wop.tile([P, NHG, D], BF16, name="wot3", tag="wo")
              with tc.tile_pool(name="khp", bufs=2) as khp, \
                 tc.tile_pool(name="vhp", bufs=2) as vhp, \
                 tc.tile_pool(name="qlq", bufs=2) as qlqp, \
                 tc.tile_pool(name="ptp", bufs=4) as ptp, \
                 tc.tile_pool(name="pap", bufs=2) as pap, \
                 tc.tile_pool(name="rcp", bufs=2) as rcp, \
                 tc.tile_pool(name="stp", bufs=4, space="PSUM") as stp, \
                 tc.tile_pool(name="otp", bufs=2, space="PSUM") as otp, \
                 tc.tile_pool(name="rsp", bufs=2, space="PSUM") as rsp:
                for h in range(NHG):
                    kb_h = kb_res[h]
                    kr_h = kr_res[h // 2][(h % 2) * DHR:(h % 2 + 1) * DHR, :]
                    vts = [v_res[:, lk, h * DH:(h + 1) * DH]
                           for lk in range(L // P)]
                    qb_h = qb_res[h]
                    qr_h = qr_res[h // 2][(h % 2) * DHR:(h % 2 + 1) * DHR, :]
                    if h == 0:
                        # prefetch W_O (bf16 from host) during attention
                        nc.sync.dma_start(out=wot3[:], in_=wo_3[:])
                    for lq in range(L // LQ):
                        qsl = slice(lq * LQ, (lq + 1) * LQ)
                        ot_ps = otp.tile([P, LQ], F32, name="ot_ps", tag="ot")
                        # two parallel accumulator chains (even/odd key tiles)
                        # so the elementwise adds don't serialize on one engine
                        accs = [pap.tile([P, LQ], F32R, name=f"pa{i}", tag=f"pa{i}")
                                for i in range(2)]
                        seen = [False, False]
                        for lk in range(L // P):
                            st_ps = stp.tile([P, LQ], F32, name="st_ps", tag="st")
                            nc.tensor.matmul(
                                st_ps[:], kb_h[:, lk * P:(lk + 1) * P],
                                qb_h[:, qsl], start=True, stop=False)
                            nc.tensor.matmul(
                                st_ps[:], kr_h[:, lk * P:(lk + 1) * P],
                                qr_h[:, qsl], start=False, stop=True)
                            pt = ptp.tile([P, LQ], BF16, name="pt", tag="pt")
                            nc.scalar.activation(
                                pt[:], st_ps[:], mybir.ActivationFunctionType.Exp,
                                scale=SCALE)
                            nc.tensor.matmul(
                                ot_ps[:], vts[lk][:], pt[:],
                                start=(lk == 0), stop=(lk == L // P - 1))
                            a = lk % 2
                            if not seen[a]:
                                nc.any.tensor_copy(accs[a][:], pt[:])
                                seen[a] = True
                            else:
                                nc.any.tensor_add(accs[a][:], accs[a][:], pt[:])
                        rs_ps = rsp.tile([P, LQ], F32, name="rs_ps", tag="rs")
                        nc.tensor.matmul(rs_ps[:], ones_t, accs[0][:],
                                         start=True, stop=False)
                        nc.tensor.matmul(rs_ps[:], ones_t, accs[1][:],
                                         start=False, stop=True)
                        rec = rcp.tile([P, LQ], F32, name="rec", tag="rec")
                        nc.vector.reciprocal(rec[:], rs_ps[:])
                        nc.any.tensor_mul(oT_res[h][:, qsl], ot_ps[:], rec[:])

              # ============= Phase C: output projection (bf16) ============
              with tc.tile_pool(name="ocp", bufs=2) as ocp, \
                   tc.tile_pool(name="psC", bufs=4, space="PSUM") as psC:
                  for st4 in range(L // P // 4):
                      oc = ocp.tile([P, 4, D], BF16, name="oc", tag="oc")
                      for g in range(4):
                          mt = st4 * 4 + g
                          for nt in range(D // 512):
                              ps = psC.tile([P, 512], F32, name="ps_o", tag="psc")
                              for k in range(NHG):
                                  nc.tensor.matmul(
                                      ps[:], oT_res[k][:, mt * P:(mt + 1) * P],
                                      wot3[:, k, nt * 512:(nt + 1) * 512],
                                      start=(k == 0), stop=(k == NHG - 1))
                              nc.any.tensor_copy(
                                  oc[:, g, nt * 512:(nt + 1) * 512], ps[:])
                      nc.sync.dma_start(
                          out=out_3[:, st4 * 4:(st4 + 1) * 4, :], in_=oc[:])

    nc.compile()
    return nc


def _rope_tables():
    """cos/sin in transposed, 2-head-replicated layout (128 x L), plus Prot^T."""
    inv_freq = 1.0 / (ROPE_THETA ** (np.arange(0, DHR, 2, dtype=np.float32) / DHR))
    ang = np.arange(L, dtype=np.float32)[:, None] * inv_freq[None, :]  # (L, 32)
    cos64 = np.concatenate([np.cos(ang), np.cos(ang)], axis=1).T  # (64, L)
    sin64 = np.concatenate([np.sin(ang), np.sin(ang)], axis=1).T
    cosr = np.ascontiguousarray(np.tile(cos64, (2, 1)), dtype=np.float32)
    sinr = np.ascontiguousarray(np.tile(sin64, (2, 1)), dtype=np.float32)
    # rot(x) = [-x2, x1] per 64-dim head: Prot rows 0:32 = -I at cols 32:64,
    # rows 32:64 = +I at cols 0:32; block-diag over 2 heads; pass transposed.
    p64 = np.zeros((DHR, DHR), dtype=np.float32)
    half = DHR // 2
    p64[np.arange(half), np.arange(half) + half] = -1.0
    p64[np.arange(half) + half, np.arange(half)] = 1.0
    p128 = np.zeros((P, P), dtype=np.float32)
    p128[:DHR, :DHR] = p64
    p128[DHR:, DHR:] = p64
    protT = np.ascontiguousarray(p128.T)
    return cosr, sinr, protT


def _make_in_maps(inputs):
    """Build the 8 per-core input maps from the full-problem input dict."""
    cosr, sinr, protT = _rope_tables()
    f = np.float32
    bf = ml_dtypes.bfloat16
    x = np.asarray(inputs["x"])
    xTs = [np.ascontiguousarray(x[b].T).astype(bf) for b in range(B)]
    # host-side query-path fold (float64 for a clean compose, cast to f32)
    wdq = np.asarray(inputs["W_D_Q"], np.float64)
    weffb_all = (wdq @ np.asarray(inputs["W_U_Q"], np.float64)).astype(f)
    weffr_all = (wdq @ np.asarray(inputs["W_Q_R"], np.float64)).astype(f)
    W_D_KV = np.ascontiguousarray(inputs["W_D_KV"], dtype=f)
    csd = np.ascontiguousarray(np.concatenate([cosr, sinr], axis=1), dtype=f)
    pod = np.ascontiguousarray(np.concatenate(
        [protT.astype(np.float32), np.ones((P, P), np.float32)], axis=1), dtype=f)
    in_maps = []
    for c in range(8):
        b, g = c // 4, c % 4
        hb = slice(g * DQB, (g + 1) * DQB)
        hr = slice(g * DQR, (g + 1) * DQR)
        weff_c = np.concatenate([weffb_all[:, hb], weffr_all[:, hr]], axis=1)
        wukv_c = np.concatenate(
            [np.asarray(inputs["W_U_K"])[:, hb], np.asarray(inputs["W_U_V"])[:, hb]],
            axis=1)
        in_maps.append(dict(
            xT=xTs[b],
            wdv=W_D_KV.astype(bf),
            wkrd=np.ascontiguousarray(np.asarray(inputs["W_K_R"])[:, hr]).astype(bf),
            weff=np.ascontiguousarray(weff_c).astype(bf),
            wukv=np.ascontiguousarray(wukv_c).astype(bf),
            wo=np.ascontiguousarray(np.asarray(inputs["W_O"])[hb, :]).astype(bf),
            csd=csd, pod=pod,
        ))
    return in_maps


def kernel(x, W_D_Q, W_U_Q, W_Q_R, W_D_KV, W_U_K, W_K_R, W_U_V, W_O):
    if "nc" not in _CACHED:
        _CACHED["nc"] = _build()
    nc = _CACHED["nc"]

    in_maps = _make_in_maps(dict(
        x=x, W_D_Q=W_D_Q, W_U_Q=W_U_Q, W_Q_R=W_Q_R, W_D_KV=W_D_KV,
        W_U_K=W_U_K, W_K_R=W_K_R, W_U_V=W_U_V, W_O=W_O))
    res = run_bass_kernel_spmd(nc, in_maps, core_ids=list(range(8)))
    outs = [np.asarray(r["out"]).astype(np.float32) for r in res.results]
    full = np.stack(
        [outs[b * 4] + outs[b * 4 + 1] + outs[b * 4 + 2] + outs[b * 4 + 3]
         for b in range(B)])
    return full
